# revision 20
# baseline (speedup 1.0000x reference)
"""BitLinear (1.58b) dense MLP kernel for 8 trn2 NeuronCores.

Computes out[b,s,o] = einsum('bsi,oi->bso', sign(x), ternarize(W)) where
ternarize(W) = sign(W/gamma) * clamp(round(|W/gamma|), max=1),
gamma = mean(|W|) + 1e-6.

Sharding: column-parallel (weight sharded along out_features across the 8
cores, x replicated). gamma needs a global reduction over W -> each core
reduces |W| over its shard on-device, a 512B AllReduce combines the partial
sums, and everything else stays local. Host only reshapes/transposes and
concatenates the output shards.

Device pipeline per core (all compute on-device):
  1. W pass 1: sum(|W_shard|) via DVE abs-reduce.  The pass is layout-
     free (a sum), so each 2MB DMA lands as one contiguous 16KB strip
     per partition -- the fastest HBM pattern.  W1 gets the full HBM
     bandwidth (everything else queues behind it).
  2. AllReduce of [128,1] partials; the DMA-idle AllReduce window
     prefetches the first two ternarize-source W pairs and the first two
     x stripes (signed on ACT).
  3. Threshold t = gamma/2 via PE ones-matmul broadcast + ACT scale.
     Identity: ternarize(W) = sign(W) * (|W| > gamma/2); strict compares
     give 0 at an exact |W| == t tie (matches round-half-even; input has
     no exact ties).
  4. Ternarize on DVE (2 ops/slab; no other engine can run compare ALU
     ops on v3), k-pair by k-pair in matmul consumption order.
  5. Warmup: PSUM only holds 2 full-width f32 groups, so while wq k-pairs
     stream in, the first 8 m-subtiles run SPLIT accumulation groups with
     staggered split points (first segment = first 4+g k-pairs).  Each
     first-segment result parks in SBUF as int16 (exact: |sum| <= 2816)
     via an ACT copy, freeing its PSUM slot at the g-th pair arrival so
     the PE always has a completed-pair backlog to chew on.  The final
     segment accumulates in PSUM and a single DVE add merges the park at
     copy-out time.
  6. Steady state: full 16-pair psum groups per m-subtile, fp8 DoubleRow
     matmuls (K=256/instr, 512-wide out = ISA max), DVE copy, DMA out.
"""

import numpy as np
from contextlib import ExitStack

import concourse.bass as bass
import concourse.bacc as bacc
import concourse.tile as tile
import concourse.mybir as mybir
from concourse.bass_utils import run_bass_kernel_spmd

N_CORES = 8
P = 128
FULL_B, FULL_S, FULL_K = 4, 2048, 4096
FULL_M = FULL_B * FULL_S       # 8192 tokens
FULL_N = 16384                 # out_features
N_SH = FULL_N // N_CORES       # 2048 per core
EPS = 1e-6

F32 = mybir.dt.float32
I16 = mybir.dt.int16
FP8 = mybir.dt.float8e4

AX = mybir.AxisListType
ALU = mybir.AluOpType
ACTF = mybir.ActivationFunctionType


def build_bitlinear(
    m_total=FULL_M,
    k_total=FULL_K,
    n_sh=N_SH,
    n_cores=N_CORES,
    n_weight_total=None,
    m_super=512,
    n_mm=512,
):
    """Inputs per core:
       xT  [k_total, m_total] f32  (sign(x) applied on device)
       wT  [k_total, n_sh]    f32  (this core's column shard of W^T)
       out [m_total, n_sh]    f32
    """
    if n_weight_total is None:
        n_weight_total = n_sh * n_cores * k_total

    KS = k_total // P              # 32 k-slabs of 128
    KP = KS // 2                   # 16 matmul k-pairs (DoubleRow)
    MS = m_total // m_super        # 16 m stripes
    MSUB = m_super // P            # 4 m-subtiles per stripe
    NB = n_sh // n_mm              # 4 psum column blocks

    assert k_total % (2 * P) == 0 and m_total % m_super == 0
    assert m_super % P == 0 and n_sh % n_mm == 0

    W_BUFS = 2                     # resident W k-pairs (16KB/partition each)
    jp_order = list(range(KP - W_BUFS, KP)) + list(range(KP - W_BUFS))
    N_WARM = 2 * MSUB              # 8 warm m-subtiles (stripes 0,1)
    SEG0 = 4                       # first warm split point; g-th warm msub
    #                                splits after SEG0+g k-pairs

    # t = gamma/2 = sum|W| * 0.5/n_total + eps/2
    scale_t = 0.5 / n_weight_total
    bias_t = 0.5 * EPS

    nc = bacc.Bacc(
        "TRN2", target_bir_lowering=False, debug=False, num_devices=n_cores
    )
    xT = nc.dram_tensor("xT", [k_total, m_total], F32, kind="ExternalInput").ap()
    wT = nc.dram_tensor("wT", [k_total, n_sh], F32, kind="ExternalInput").ap()
    out = nc.dram_tensor("out", [m_total, n_sh], F32, kind="ExternalOutput").ap()

    dr = mybir.MatmulPerfMode.DoubleRow

    with tile.TileContext(nc) as tc, ExitStack() as ctx:
        consts = ctx.enter_context(tc.tile_pool(name="consts", bufs=1))
        wqp = ctx.enter_context(tc.tile_pool(name="wqp", bufs=1))
        wstage = ctx.enter_context(tc.tile_pool(name="wstage", bufs=W_BUFS))
        bvp = ctx.enter_context(tc.tile_pool(name="bvp", bufs=4))
        rdp = ctx.enter_context(tc.tile_pool(name="rdp", bufs=2))
        redp = ctx.enter_context(tc.tile_pool(name="redp", bufs=1))
        xstage = ctx.enter_context(tc.tile_pool(name="xstage", bufs=2))
        xsp = ctx.enter_context(tc.tile_pool(name="xsp", bufs=2))
        partp = ctx.enter_context(tc.tile_pool(name="partp", bufs=N_WARM))
        outp = ctx.enter_context(tc.tile_pool(name="outp", bufs=2))
        psum = ctx.enter_context(tc.tile_pool(name="psum", bufs=2, space="PSUM"))
        ccp = ctx.enter_context(tc.tile_pool(name="ccp", bufs=1, space="DRAM"))

        ones = consts.tile([P, P], F32)
        nc.vector.memset(ones, 1.0)

        # ---- phase 1: local sum(|W|) over 2MB pair-slab DMAs ----
        RCH = 128
        n_r = 2 * n_sh // RCH      # 32 reduce chunks per pair-slab
        wf_resident = {}
        partials = redp.tile([P, KP], F32)
        for jp in range(KP):
            wf2 = wstage.tile([P, 2, n_sh], F32, name="wf2", tag="wf2")
            src = wT[2 * jp * P : (2 * jp + 2) * P, :].rearrange(
                "(h p) d -> p h d", p=P
            )
            nc.sync.dma_start(wf2, src)
            if jp >= KP - W_BUFS:
                wf_resident[jp] = wf2
            r32 = rdp.tile([P, n_r], F32, name="r32", tag="r32")
            nc.vector.tensor_reduce(
                r32, wf2.rearrange("p h (c r) -> p (h c) r", r=RCH), axis=AX.X,
                op=ALU.add, apply_absolute_value=True,
            )
            nc.vector.tensor_reduce(
                partials[:, jp : jp + 1], r32, axis=AX.X, op=ALU.add
            )
        p_loc = redp.tile([P, 1], F32)
        nc.vector.tensor_reduce(p_loc, partials, axis=AX.X, op=ALU.add)

        # ---- phase 2: AllReduce the [128,1] partials ----
        cc_in = ccp.tile([P, 1], F32)
        cc_out = ccp.tile([P, 1], F32, addr_space="Shared")
        nc.sync.dma_start(cc_in, p_loc)
        nc.gpsimd.collective_compute(
            "AllReduce", ALU.add,
            replica_groups=[list(range(n_cores))],
            ins=[cc_in], outs=[cc_out],
        )
        p_glob = redp.tile([P, 1], F32)
        nc.sync.dma_start(p_glob, cc_out)

        # ---- x prefetch: stripes 0,1 (1MB 2-pair DMAs) during the
        # AllReduce window (queued after W1 + cc_in), signed on ACT.
        xgroups = []
        for jp in jp_order:
            g = jp & ~1
            if g not in xgroups:
                xgroups.append(g)

        def load_stripe(ms):
            xs = xsp.tile([P, KP, 2, m_super], FP8, name="xs")
            for g in xgroups:
                xf = xstage.tile([P, 4, m_super], F32, name="xf")
                src = xT[
                    2 * g * P : (2 * g + 4) * P,
                    ms * m_super : (ms + 1) * m_super,
                ].rearrange("(n p) d -> p n d", p=P)
                nc.sync.dma_start(xf, src)
                nc.scalar.sign(xs[:, g : g + 2, :, :], xf)
            return xs

        xs_pre = {ms: load_stripe(ms) for ms in range(2)}

        # ---- phase 3: threshold t broadcast to all partitions ----
        gps = psum.tile([P, n_mm], F32, name="gps", tag="ps")
        nc.tensor.matmul(gps[:, 0:1], lhsT=ones, rhs=p_glob, start=True, stop=True)
        t_pos = redp.tile([P, 1], F32)
        t_neg = redp.tile([P, 1], F32)
        nc.scalar.activation(t_pos, gps[:, 0:1], ACTF.Copy, bias=bias_t, scale=scale_t)
        nc.scalar.activation(t_neg, gps[:, 0:1], ACTF.Copy, bias=-bias_t, scale=-scale_t)

        # ---- phase 4: ternarize W -> wq in {-1,0,1} fp8 (DVE) ----
        # wq = (w > t) - (w < -t); strict compares give 0 at an exact
        # |w| == t tie, matching round-half-even.
        wq = wqp.tile([P, KP, 2, n_sh], FP8)
        for jp in jp_order:
            wf2 = wf_resident.get(jp)
            if wf2 is None:
                wf2 = wstage.tile([P, 2, n_sh], F32, name="wf2", tag="wf2")
                src = wT[2 * jp * P : (2 * jp + 2) * P, :].rearrange(
                    "(h p) d -> p h d", p=P
                )
                nc.sync.dma_start(wf2, src)
            for half in range(2):
                b = bvp.tile([P, n_sh], FP8, name="b", tag="b")
                nc.vector.tensor_scalar(b, wf2[:, half, :], t_neg, None, op0=ALU.is_lt)
                nc.vector.scalar_tensor_tensor(
                    wq[:, jp, half, :], wf2[:, half, :], t_pos, b,
                    op0=ALU.is_gt, op1=ALU.subtract,
                )

        # ---- phases 5+6: matmuls ----
        def emit_mms(ps, xs, msub, jp, start, stop):
            lhsT = xs[:, jp, :, msub * P : (msub + 1) * P]
            for nb in range(NB):
                nc.tensor.matmul(
                    ps[:, nb * n_mm : (nb + 1) * n_mm],
                    lhsT,
                    wq[:, jp, :, nb * n_mm : (nb + 1) * n_mm],
                    start=start,
                    stop=stop,
                    perf_mode=dr,
                )

        # Warmup m-subtiles 0..7 (stripes 0,1): split accumulation groups
        # with staggered split points.  First segment (jp_order[:SEG0+g])
        # parks as int16; psum slots recycle at successive pair arrivals.
        parks = []
        for g in range(N_WARM):
            cut = SEG0 + g
            ps = psum.tile([P, n_sh], F32, name="ps", tag="ps")
            for idx, jp in enumerate(jp_order[:cut]):
                emit_mms(
                    ps, xs_pre[g // MSUB], g % MSUB, jp,
                    start=(idx == 0), stop=(idx == cut - 1),
                )
            pt = partp.tile([P, n_sh], I16, name="pt")
            nc.scalar.copy(pt, ps)
            parks.append(pt)

        # Final segments + merge for the warm m-subtiles.
        for g in range(N_WARM):
            cut = SEG0 + g
            ps = psum.tile([P, n_sh], F32, name="ps", tag="ps")
            for idx, jp in enumerate(jp_order[cut:]):
                emit_mms(
                    ps, xs_pre[g // MSUB], g % MSUB, jp,
                    start=(idx == 0), stop=(idx == KP - cut - 1),
                )
            ot = outp.tile([P, n_sh], F32, name="ot", tag="ot")
            nc.vector.tensor_tensor(ot, ps, parks[g], op=ALU.add)
            nc.sync.dma_start(out[g * P : (g + 1) * P, :], ot)

        # Steady state: full-depth groups for stripes 2..MS-1.
        for ms in range(2, MS):
            xs = load_stripe(ms)
            for msub in range(MSUB):
                ps = psum.tile([P, n_sh], F32, name="ps", tag="ps")
                for idx, jp in enumerate(jp_order):
                    emit_mms(
                        ps, xs, msub, jp,
                        start=(idx == 0), stop=(idx == KP - 1),
                    )
                ot = outp.tile([P, n_sh], F32, name="ot", tag="ot")
                nc.vector.tensor_copy(ot, ps)
                m_row = (ms * MSUB + msub) * P
                nc.sync.dma_start(out[m_row : m_row + P, :], ot)

    nc.compile()
    return nc


_NC_CACHE = {}


def _get_nc():
    if "full" not in _NC_CACHE:
        _NC_CACHE["full"] = build_bitlinear()
    return _NC_CACHE["full"]


def kernel(x: np.ndarray, weight: np.ndarray) -> np.ndarray:
    assert x.shape == (FULL_B, FULL_S, FULL_K) and weight.shape == (FULL_N, FULL_K)
    x = np.ascontiguousarray(x, dtype=np.float32)
    weight = np.ascontiguousarray(weight, dtype=np.float32)

    # Host-side layout prep only: transpose to [K, M] / [K, N] and slice the
    # column shards. All arithmetic happens on-device.
    xT = np.ascontiguousarray(x.reshape(FULL_M, FULL_K).T)
    wT_full = weight.T  # [K, N] view
    in_maps = []
    for c in range(N_CORES):
        wT_sh = np.ascontiguousarray(wT_full[:, c * N_SH : (c + 1) * N_SH])
        in_maps.append({"xT": xT, "wT": wT_sh})

    nc = _get_nc()
    res = run_bass_kernel_spmd(nc, in_maps, core_ids=list(range(N_CORES)))
    out = np.concatenate([res.results[c]["out"] for c in range(N_CORES)], axis=1)
    return out.reshape(FULL_B, FULL_S, FULL_N).astype(np.float32)


# revision 22
# speedup vs baseline: 1.0619x; 1.0619x over previous
"""BitLinear (1.58b) dense MLP kernel for 8 trn2 NeuronCores.

Computes out[b,s,o] = einsum('bsi,oi->bso', sign(x), ternarize(W)) where
ternarize(W) = sign(W/gamma) * clamp(round(|W/gamma|), max=1),
gamma = mean(|W|) + 1e-6.

Sharding: column-parallel (weight sharded along out_features across the 8
cores, x replicated). gamma needs a global reduction over W -> each core
reduces |W| over its shard on-device, a 512B AllReduce combines the partial
sums, and everything else stays local. Host only reshapes/transposes and
concatenates the output shards.

Device pipeline per core (all compute on-device):
  1. W pass 1: sum(|W_shard|) via DVE abs-reduce.  The pass is layout-
     free (a sum), so each 2MB DMA lands as one contiguous 16KB strip
     per partition -- the fastest HBM pattern.  W1 gets the full HBM
     bandwidth (everything else queues behind it).
  2. AllReduce of [128,1] partials; the DMA-idle AllReduce window
     prefetches the first two ternarize-source W pairs and the first two
     x stripes (signed on ACT).
  3. Threshold t = gamma/2 via PE ones-matmul broadcast + ACT scale.
     Identity: ternarize(W) = sign(W) * (|W| > gamma/2); strict compares
     give 0 at an exact |W| == t tie (matches round-half-even; input has
     no exact ties).
  4. Ternarize on DVE (2 ops/slab; no other engine can run compare ALU
     ops on v3), k-pair by k-pair in matmul consumption order.
  5. Warmup: PSUM only holds 2 full-width f32 groups, so while wq k-pairs
     stream in, the first 8 m-subtiles run SPLIT accumulation groups with
     staggered split points (first segment = first 4+g k-pairs).  Each
     first-segment result parks in SBUF as int16 (exact: |sum| <= 2816)
     via an ACT copy, freeing its PSUM slot at the g-th pair arrival so
     the PE always has a completed-pair backlog to chew on.  The final
     segment accumulates in PSUM and a single DVE add merges the park at
     copy-out time.
  6. Steady state: full 16-pair psum groups per m-subtile, fp8 DoubleRow
     matmuls (K=256/instr, 512-wide out = ISA max), DVE copy, DMA out.
"""

import numpy as np
from contextlib import ExitStack

import concourse.bass as bass
import concourse.bacc as bacc
import concourse.tile as tile
import concourse.mybir as mybir
from concourse.bass_utils import run_bass_kernel_spmd

N_CORES = 8
P = 128
FULL_B, FULL_S, FULL_K = 4, 2048, 4096
FULL_M = FULL_B * FULL_S       # 8192 tokens
FULL_N = 16384                 # out_features
N_SH = FULL_N // N_CORES       # 2048 per core
EPS = 1e-6

F32 = mybir.dt.float32
I16 = mybir.dt.int16
FP8 = mybir.dt.float8e4

AX = mybir.AxisListType
ALU = mybir.AluOpType
ACTF = mybir.ActivationFunctionType


def build_bitlinear(
    m_total=FULL_M,
    k_total=FULL_K,
    n_sh=N_SH,
    n_cores=N_CORES,
    n_weight_total=None,
    m_super=512,
    n_mm=512,
):
    """Inputs per core:
       xT  [k_total, m_total] f32  (sign(x) applied on device)
       wT  [k_total, n_sh]    f32  (this core's column shard of W^T)
       out [m_total, n_sh]    f32
    """
    if n_weight_total is None:
        n_weight_total = n_sh * n_cores * k_total

    KS = k_total // P              # 32 k-slabs of 128
    KP = KS // 2                   # 16 matmul k-pairs (DoubleRow)
    MS = m_total // m_super        # 16 m stripes
    MSUB = m_super // P            # 4 m-subtiles per stripe
    NB = n_sh // n_mm              # 4 psum column blocks

    assert k_total % (2 * P) == 0 and m_total % m_super == 0
    assert m_super % P == 0 and n_sh % n_mm == 0

    W_BUFS = 2                     # resident W k-pairs (16KB/partition each)
    jp_order = list(range(KP - W_BUFS, KP)) + list(range(KP - W_BUFS))
    N_WARM = 2 * MSUB              # 8 warm m-subtiles (stripes 0,1)
    SEG0 = 4                       # first warm split point; g-th warm msub
    #                                splits after SEG0+g k-pairs

    # t = gamma/2 = sum|W| * 0.5/n_total + eps/2
    scale_t = 0.5 / n_weight_total
    bias_t = 0.5 * EPS

    nc = bacc.Bacc(
        "TRN2", target_bir_lowering=False, debug=False, num_devices=n_cores
    )
    xT = nc.dram_tensor("xT", [k_total, m_total], F32, kind="ExternalInput").ap()
    wT = nc.dram_tensor("wT", [k_total, n_sh], F32, kind="ExternalInput").ap()
    out = nc.dram_tensor("out", [m_total, n_sh], F32, kind="ExternalOutput").ap()

    dr = mybir.MatmulPerfMode.DoubleRow

    with tile.TileContext(nc) as tc, ExitStack() as ctx:
        consts = ctx.enter_context(tc.tile_pool(name="consts", bufs=1))
        wqp = ctx.enter_context(tc.tile_pool(name="wqp", bufs=1))
        wstage = ctx.enter_context(tc.tile_pool(name="wstage", bufs=W_BUFS))
        bvp = ctx.enter_context(tc.tile_pool(name="bvp", bufs=4))
        rdp = ctx.enter_context(tc.tile_pool(name="rdp", bufs=2))
        redp = ctx.enter_context(tc.tile_pool(name="redp", bufs=1))
        xstage = ctx.enter_context(tc.tile_pool(name="xstage", bufs=2))
        xsp = ctx.enter_context(tc.tile_pool(name="xsp", bufs=2))
        partp = ctx.enter_context(tc.tile_pool(name="partp", bufs=N_WARM))
        outp = ctx.enter_context(tc.tile_pool(name="outp", bufs=2))
        psum = ctx.enter_context(tc.tile_pool(name="psum", bufs=2, space="PSUM"))
        ccp = ctx.enter_context(tc.tile_pool(name="ccp", bufs=1, space="DRAM"))

        ones = consts.tile([P, P], F32)
        nc.vector.memset(ones, 1.0)

        # ---- phase 1: local sum(|W|) over 2MB pair-slab DMAs ----
        RCH = 128
        n_r = 2 * n_sh // RCH      # 32 reduce chunks per pair-slab
        wf_resident = {}
        partials = redp.tile([P, KP], F32)
        for jp in range(KP):
            wf2 = wstage.tile([P, 2, n_sh], F32, name="wf2", tag="wf2")
            src = wT[2 * jp * P : (2 * jp + 2) * P, :].rearrange(
                "(h p) d -> p h d", p=P
            )
            nc.sync.dma_start(wf2, src)
            if jp >= KP - W_BUFS:
                wf_resident[jp] = wf2
            r32 = rdp.tile([P, n_r], F32, name="r32", tag="r32")
            nc.vector.tensor_reduce(
                r32, wf2.rearrange("p h (c r) -> p (h c) r", r=RCH), axis=AX.X,
                op=ALU.add, apply_absolute_value=True,
            )
            nc.vector.tensor_reduce(
                partials[:, jp : jp + 1], r32, axis=AX.X, op=ALU.add
            )
        p_loc = redp.tile([P, 1], F32)
        nc.vector.tensor_reduce(p_loc, partials, axis=AX.X, op=ALU.add)

        # ---- phase 2: AllReduce the [128,1] partials ----
        cc_in = ccp.tile([P, 1], F32)
        cc_out = ccp.tile([P, 1], F32, addr_space="Shared")
        nc.sync.dma_start(cc_in, p_loc)
        nc.gpsimd.collective_compute(
            "AllReduce", ALU.add,
            replica_groups=[list(range(n_cores))],
            ins=[cc_in], outs=[cc_out],
        )
        p_glob = redp.tile([P, 1], F32)
        nc.sync.dma_start(p_glob, cc_out)

        # ---- x prefetch: stripes 0,1 (1MB 2-pair DMAs) during the
        # AllReduce window (queued after W1 + cc_in), signed on ACT.
        xgroups = []
        for jp in jp_order:
            g = jp & ~1
            if g not in xgroups:
                xgroups.append(g)

        def load_stripe(ms):
            xs = xsp.tile([P, KP, 2, m_super], FP8, name="xs")
            for g in xgroups:
                xf = xstage.tile([P, 4, m_super], F32, name="xf")
                src = xT[
                    2 * g * P : (2 * g + 4) * P,
                    ms * m_super : (ms + 1) * m_super,
                ].rearrange("(n p) d -> p n d", p=P)
                nc.sync.dma_start(xf, src)
                nc.scalar.sign(xs[:, g : g + 2, :, :], xf)
            return xs

        xs_pre = {ms: load_stripe(ms) for ms in range(2)}

        # ---- phase 3: threshold t broadcast to all partitions ----
        gps = psum.tile([P, n_mm], F32, name="gps", tag="ps")
        nc.tensor.matmul(gps[:, 0:1], lhsT=ones, rhs=p_glob, start=True, stop=True)
        t_pos = redp.tile([P, 1], F32)
        t_neg = redp.tile([P, 1], F32)
        nc.scalar.activation(t_pos, gps[:, 0:1], ACTF.Copy, bias=bias_t, scale=scale_t)
        nc.scalar.activation(t_neg, gps[:, 0:1], ACTF.Copy, bias=-bias_t, scale=-scale_t)

        # ---- phase 4: ternarize W -> wq in {-1,0,1} fp8 (DVE) ----
        # wq = (w > t) - (w < -t); strict compares give 0 at an exact
        # |w| == t tie, matching round-half-even.
        wq = wqp.tile([P, KP, 2, n_sh], FP8)
        for jp in jp_order:
            wf2 = wf_resident.get(jp)
            if wf2 is None:
                wf2 = wstage.tile([P, 2, n_sh], F32, name="wf2", tag="wf2")
                src = wT[2 * jp * P : (2 * jp + 2) * P, :].rearrange(
                    "(h p) d -> p h d", p=P
                )
                nc.sync.dma_start(wf2, src)
            for half in range(2):
                b = bvp.tile([P, n_sh], FP8, name="b", tag="b")
                nc.vector.tensor_scalar(b, wf2[:, half, :], t_neg, None, op0=ALU.is_lt)
                nc.vector.scalar_tensor_tensor(
                    wq[:, jp, half, :], wf2[:, half, :], t_pos, b,
                    op0=ALU.is_gt, op1=ALU.subtract,
                )

        # ---- phases 5+6: matmuls ----
        def emit_mms(ps, xs, msub, jp, start, stop):
            lhsT = xs[:, jp, :, msub * P : (msub + 1) * P]
            for nb in range(NB):
                nc.tensor.matmul(
                    ps[:, nb * n_mm : (nb + 1) * n_mm],
                    lhsT,
                    wq[:, jp, :, nb * n_mm : (nb + 1) * n_mm],
                    start=start,
                    stop=stop,
                    perf_mode=dr,
                )

        # Warmup m-subtiles 0..7 (stripes 0,1): split accumulation groups
        # with staggered split points.  First segment (jp_order[:SEG0+g])
        # parks as int16; psum slots recycle at successive pair arrivals.
        parks = []
        for g in range(N_WARM):
            cut = SEG0 + g
            ps = psum.tile([P, n_sh], F32, name="ps", tag="ps")
            for idx, jp in enumerate(jp_order[:cut]):
                emit_mms(
                    ps, xs_pre[g // MSUB], g % MSUB, jp,
                    start=(idx == 0), stop=(idx == cut - 1),
                )
            pt = partp.tile([P, n_sh], I16, name="pt")
            nc.scalar.copy(pt, ps)
            parks.append(pt)

        # Final segments + merge for the warm m-subtiles.
        for g in range(N_WARM):
            cut = SEG0 + g
            ps = psum.tile([P, n_sh], F32, name="ps", tag="ps")
            for idx, jp in enumerate(jp_order[cut:]):
                emit_mms(
                    ps, xs_pre[g // MSUB], g % MSUB, jp,
                    start=(idx == 0), stop=(idx == KP - cut - 1),
                )
            ot = outp.tile([P, n_sh], F32, name="ot", tag="ot")
            nc.vector.tensor_tensor(ot, ps, parks[g], op=ALU.add)
            # Output stores go out on the ACT HWDGE ring (qActDynamicHW) so
            # they never block input loads on the sync ring's in-order FIFO
            # (the warm outputs otherwise stall the next stripes' x loads).
            nc.scalar.dma_start(out[g * P : (g + 1) * P, :], ot)

        # Steady state: full-depth groups for stripes 2..MS-1.
        for ms in range(2, MS):
            xs = load_stripe(ms)
            for msub in range(MSUB):
                ps = psum.tile([P, n_sh], F32, name="ps", tag="ps")
                for idx, jp in enumerate(jp_order):
                    emit_mms(
                        ps, xs, msub, jp,
                        start=(idx == 0), stop=(idx == KP - 1),
                    )
                ot = outp.tile([P, n_sh], F32, name="ot", tag="ot")
                nc.vector.tensor_copy(ot, ps)
                m_row = (ms * MSUB + msub) * P
                nc.scalar.dma_start(out[m_row : m_row + P, :], ot)

    nc.compile()
    return nc


_NC_CACHE = {}


def _get_nc():
    if "full" not in _NC_CACHE:
        _NC_CACHE["full"] = build_bitlinear()
    return _NC_CACHE["full"]


def kernel(x: np.ndarray, weight: np.ndarray) -> np.ndarray:
    assert x.shape == (FULL_B, FULL_S, FULL_K) and weight.shape == (FULL_N, FULL_K)
    x = np.ascontiguousarray(x, dtype=np.float32)
    weight = np.ascontiguousarray(weight, dtype=np.float32)

    # Host-side layout prep only: transpose to [K, M] / [K, N] and slice the
    # column shards. All arithmetic happens on-device.
    xT = np.ascontiguousarray(x.reshape(FULL_M, FULL_K).T)
    wT_full = weight.T  # [K, N] view
    in_maps = []
    for c in range(N_CORES):
        wT_sh = np.ascontiguousarray(wT_full[:, c * N_SH : (c + 1) * N_SH])
        in_maps.append({"xT": xT, "wT": wT_sh})

    nc = _get_nc()
    res = run_bass_kernel_spmd(nc, in_maps, core_ids=list(range(N_CORES)))
    out = np.concatenate([res.results[c]["out"] for c in range(N_CORES)], axis=1)
    return out.reshape(FULL_B, FULL_S, FULL_N).astype(np.float32)
